# revision 10
# baseline (speedup 1.0000x reference)
"""Trainium2 Bass kernel for causal single-head attention with QKV projections.

Problem shape: B=4, S=4096, E=512, H=64 (fp32 inputs, causal mask).

Strategy (8 NeuronCores, data-parallel):
  - core j handles batch j%4; half j//4 of that batch's query rows.
    Half 0 = q-blocks {0,1,6,7}, half 1 = {2,3,4,5} (512-row blocks), so both
    halves own exactly 72 causal key-chunks -> balanced Tensor AND Scalar
    (exp) load, and the in-variant pairs (0,1),(2,3) have near-equal chunk
    counts -> minimal single-stream softmax rounds (40 exp instructions).
  - Host pre-transposes Q/K/V slabs to [E, S] layout and casts to bf16 so all
    device matmuls have the contraction dim on partitions.
  - On device: project Qt=[2H,Sq], Kt=[2H,S] (weights host-duplicated to
    2H=128 so score matmuls of a stream pair run concurrently in disjoint
    64-row PE groups), and V directly into [s,h] layout (stationary = vT
    chunk, moving = Wv[e,64]; 8 s-chunks packed per PSUM bank; bias added
    during the DVE evacuation) - no PE transposes for V.
  - Flash-style causal attention, scores transposed [k-part, q-free]:
       St = Kt_chunk^T @ Qt ; exp fused into the PSUM->SBUF evacuation
       O^T (+denominator row) = [v | 1 | 0pad]^T @ P accumulated in PSUM
    No max-subtraction (|s|<~1 after 1/sqrt(E) scaling, softmax is
    shift-invariant).
  - Per 512-query block epilogue: PE-transpose O^T back, normalize by the
    denominator, DMA out fp32 [Sq, H].
  - Projections are interleaved with attention rounds so PE/ACT chase the
    K/V DMA stream; a tc.If on partition_id picks the variant.
"""

import sys

sys.path.insert(0, "/opt/trn_rl_repo")

import math

import numpy as np
import ml_dtypes

B, S, E, H = 4, 4096, 512, 64
N_CORES = 8
SQ = S // 2  # 2048 query rows per core
JBLK = 512  # query block size
NJ = SQ // JBLK  # 4 query blocks per core
KCH = 128  # key chunk size
JGLOBALS = [[0, 1, 6, 7], [2, 3, 4, 5]]  # global 512-row q-block ids per half
KVEXT = [8, 6]  # 512-col K/V blocks each variant actually reads
SCALE = 1.0 / math.sqrt(float(E))

BF16 = ml_dtypes.bfloat16

_CACHE = {}


def _build():
    import concourse.mybir as mybir
    from concourse import bacc, tile

    f32 = mybir.dt.float32
    bf16 = mybir.dt.bfloat16

    nc = bacc.Bacc(
        "TRN2", target_bir_lowering=False, debug=False, num_devices=N_CORES
    )

    qT = nc.dram_tensor("qT", [E, SQ], bf16, kind="ExternalInput")
    kT = nc.dram_tensor("kT", [E, S], bf16, kind="ExternalInput")
    vT = nc.dram_tensor("vT", [E, S], bf16, kind="ExternalInput")
    # weights/biases pre-swizzled on host to their SBUF layouts so the DMAs
    # are contiguous per partition (no descriptor storms)
    wTp = nc.dram_tensor("wTp", [128, 3, 4, 2 * H], bf16, kind="ExternalInput")
    bql = nc.dram_tensor("bql", [2 * H, 3], f32, kind="ExternalInput")
    bvrep = nc.dram_tensor("bvrep", [128, 4, H], f32, kind="ExternalInput")
    out = nc.dram_tensor("out", [SQ, H], f32, kind="ExternalOutput")

    # Embedded constants: causal block mask (allowed = k <= q) and identity.
    tril_np = np.triu(np.ones((KCH, KCH), np.float32)).astype(BF16)
    identf_np = np.eye(128, dtype=np.float32)
    trilc = nc.inline_tensor(tril_np, name="trilc")
    identfc = nc.inline_tensor(identf_np, name="identfc")

    with tile.TileContext(nc) as tc:
        pid = nc.partition_id()
        with (
            tc.tile_pool(name="cpool", bufs=1) as cpool,
            tc.tile_pool(name="ipool", bufs=1) as ipool,
        ):
            # ---- constants (contiguous DMAs; host pre-swizzled) ----
            w_sb = cpool.tile([128, 3, 4, 2 * H], bf16, name="w_sb")
            nc.sync.dma_start(w_sb[:], wTp.ap())
            b_sb = cpool.tile([2 * H, 3], f32, name="b_sb")
            nc.sync.dma_start(b_sb[:], bql.ap())
            bvb_sb = cpool.tile([128, 4, H], f32, name="bvb_sb")
            nc.sync.dma_start(bvb_sb[:], bvrep.ap())
            tril_sb = cpool.tile([KCH, KCH], bf16, name="tril_sb")
            nc.sync.dma_start(tril_sb[:], trilc.ap())
            identf_sb = cpool.tile([128, 128], f32, name="identf_sb")
            nc.sync.dma_start(identf_sb[:], identfc.ap())
            zbias = cpool.tile([128, 1], f32, name="zbias")
            nc.vector.memset(zbias[:], 0.0)

            # ---- input DMAs, split across three issue queues ----
            # scalar: qT halves (the exp engine is idle during the ramp);
            # sync: K stream; gpsimd: V stream (+ quarter 3 inside variant 0,
            # + output).  First two K/V blocks are issued at 512-col grain so
            # the first attention rounds start as early as possible.
            qT_sb = [
                ipool.tile([128, SQ], bf16, name=f"qT{c}", tag=f"qT{c}")
                for c in range(4)
            ]
            kT_sb = [
                ipool.tile([128, 4, S // 4], bf16, name=f"kT{c}", tag=f"kT{c}")
                for c in range(4)
            ]
            vT_sb = [
                ipool.tile([128, 4, S // 4], bf16, name=f"vT{c}", tag=f"vT{c}")
                for c in range(4)
            ]
            for half in range(2):
                for c in range(4):
                    nc.scalar.dma_start(
                        qT_sb[c][:, 1024 * half : 1024 * (half + 1)],
                        qT.ap()[
                            128 * c : 128 * (c + 1),
                            1024 * half : 1024 * (half + 1),
                        ],
                    )

            def dma_kv_block(eng, srcd, dst, blk):
                for c in range(4):
                    qt, off = divmod(512 * blk, S // 4)
                    eng.dma_start(
                        dst[c][:, qt, off : off + 512],
                        srcd.ap()[
                            128 * c : 128 * (c + 1), 512 * blk : 512 * (blk + 1)
                        ],
                    )

            def dma_kv_quarter(eng, srcd, dst, qt):
                for c in range(4):
                    eng.dma_start(
                        dst[c][:, qt, :],
                        srcd.ap()[
                            128 * c : 128 * (c + 1),
                            (S // 4) * qt : (S // 4) * (qt + 1),
                        ],
                    )

            dma_kv_block(nc.sync, kT, kT_sb, 0)
            dma_kv_block(nc.gpsimd, vT, vT_sb, 0)
            dma_kv_block(nc.sync, kT, kT_sb, 1)
            dma_kv_block(nc.gpsimd, vT, vT_sb, 1)
            for qt in range(1, 3):
                dma_kv_quarter(nc.sync, kT, kT_sb, qt)
                dma_kv_quarter(nc.gpsimd, vT, vT_sb, qt)

            def body(jglobals, vtag):
                """Whole per-core pipeline for one causal-structure variant:
                projections interleaved with the longer stream-pair's
                attention rounds (round r needs exactly key chunk r, which
                projection block r//4 produces), then the shorter pair."""
                if vtag == 0:
                    dma_kv_quarter(nc.gpsimd, kT, kT_sb, 3)
                    dma_kv_quarter(nc.gpsimd, vT, vT_sb, 3)
                with (
                    tc.tile_pool(name=f"bpool{vtag}", bufs=1) as bpool,
                    tc.tile_pool(name=f"bps{vtag}", bufs=1, space="PSUM") as bps,
                ):
                    Qt = bpool.tile([2 * H, SQ], bf16, name=f"Qt{vtag}")
                    Kt = bpool.tile([2 * H, S], bf16, name=f"Kt{vtag}")
                    v_sb = bpool.tile(
                        [128, S // KCH, 128], bf16, name=f"v_sb{vtag}"
                    )
                    # ones column = softmax denominator row; zero padding so
                    # the [128,128] PV stationary adds nothing above row H
                    nc.vector.memset(v_sb[:, :, H : H + 1], 1.0)
                    nc.vector.memset(v_sb[:, :, H + 1 :], 0.0)

                    def proj_block(dst, srcs, m, blk):
                        ps = bps.tile(
                            [2 * H, 512], f32, name=f"pj{vtag}_{m}_{blk}",
                            tag="proj", bufs=2,
                        )
                        qt, off = divmod(512 * blk, S // 4)
                        for c in range(4):
                            nc.tensor.matmul(
                                ps[:],
                                w_sb[:, m, c, :],
                                srcs[c][:, qt, off : off + 512]
                                if len(srcs[c].shape) == 3
                                else srcs[c][:, 512 * blk : 512 * (blk + 1)],
                                start=(c == 0),
                                stop=(c == 3),
                            )
                        nc.vector.tensor_scalar_add(
                            dst[:, 512 * blk : 512 * (blk + 1)],
                            ps[:],
                            b_sb[:, m : m + 1],
                        )

                    def vproj_block(vb):
                        """Project V straight into [s,h] layout: stationary is
                        the raw vT chunk, moving is Wv[e,:H]; 4 s-chunks of the
                        512-col block share one PSUM bank; the DVE evacuation
                        adds the bias and writes bf16 into v_sb."""
                        vps = bps.tile(
                            [128, 512], f32, name=f"vp{vtag}_{vb}", tag="proj",
                            bufs=2,
                        )
                        for i in range(4):
                            ci = 4 * vb + i
                            qt, off = divmod(128 * ci, S // 4)
                            for c in range(4):
                                nc.tensor.matmul(
                                    vps[:, 128 * i : 128 * i + H],
                                    vT_sb[c][:, qt, off : off + 128],
                                    w_sb[:, 2, c, 0:H],
                                    start=(c == 0),
                                    stop=(c == 3),
                                )
                        nc.vector.tensor_add(
                            v_sb[:, 4 * vb : 4 * (vb + 1), 0:H],
                            vps[:].rearrange("p (c h) -> p c h", c=4)[:, :, 0:H],
                            bvb_sb[:],
                        )

                    def chunk_geom(nk, ki):
                        d = ki - (nk - 4)  # >=0 for the 4 diagonal chunks
                        qlo = 0 if d < 0 else KCH * d
                        return d, qlo

                    def emit_st_pair(st8, pair, ki):
                        active = [x for x in pair if ki < st8[x]["nk"]]
                        st2 = bps.tile(
                            [128, 2 * JBLK], f32,
                            name=f"st{vtag}_{pair[0]}_{ki}", tag="st", bufs=2,
                        )
                        p2 = bpool.tile(
                            [128, 2 * JBLK], bf16,
                            name=f"p{vtag}_{pair[0]}_{ki}", tag="p", bufs=12,
                        )
                        diag = []
                        span = []
                        # the two streams' score matmuls run concurrently in
                        # disjoint PE row groups (Kt/Qt rows 64..127 hold the
                        # duplicated head dim, so row group 1 reads the copy)
                        for idx, x in enumerate(active):
                            s = st8[x]
                            d, qlo = chunk_geom(s["nk"], ki)
                            off = JBLK * (x - pair[0])
                            rg = 64 * idx
                            nc.tensor.matmul(
                                st2[:, off + qlo : off + JBLK],
                                Kt[rg : rg + H, KCH * ki : KCH * (ki + 1)],
                                Qt[
                                    rg : rg + H,
                                    JBLK * s["jl"] + qlo : JBLK * (s["jl"] + 1),
                                ],
                                start=True,
                                stop=True,
                                tile_position=(rg, 0),
                            )
                            span.append((off + qlo, off + JBLK))
                            if d >= 0:
                                diag.append(off + qlo)
                        lo, hi = span[0][0], span[-1][1]
                        nc.scalar.activation(
                            p2[:, lo:hi],
                            st2[:, lo:hi],
                            mybir.ActivationFunctionType.Exp,
                            bias=zbias[:],
                            scale=float(SCALE),
                        )
                        for off in diag:
                            nc.vector.tensor_mul(
                                p2[:, off : off + KCH], p2[:, off : off + KCH],
                                tril_sb[:],
                            )
                        return p2

                    def emit_pv(st8, pair, x, ki, p2):
                        s = st8[x]
                        d, qlo = chunk_geom(s["nk"], ki)
                        off = JBLK * (x - pair[0])
                        nc.tensor.matmul(
                            s["ot"][:, qlo:JBLK],
                            v_sb[:, ki, :],
                            p2[:, off + qlo : off + JBLK],
                            start=(ki == 0),
                            stop=(ki == s["nk"] - 1),
                        )

                    def epilogue(ot, jl):
                        otf = bpool.tile(
                            [H + 1, JBLK], f32, name=f"otf{vtag}_{jl}", tag="otf",
                            bufs=2,
                        )
                        nc.vector.tensor_copy(otf[:], ot[0 : H + 1, :])
                        ost = bpool.tile(
                            [128, 4, H], f32, name=f"ost{vtag}_{jl}", tag="ost",
                            bufs=2,
                        )
                        for t in range(4):
                            otr = bps.tile(
                                [128, H + 1], f32, name=f"otr{vtag}_{jl}_{t}",
                                tag="st", bufs=2,
                            )
                            nc.tensor.transpose(
                                otr[:],
                                otf[:, 128 * t : 128 * (t + 1)],
                                identf_sb[0 : H + 1, 0 : H + 1],
                            )
                            rec = bpool.tile(
                                [128, 1], f32, name=f"rec{vtag}_{jl}_{t}",
                                tag="rec", bufs=2,
                            )
                            nc.vector.reciprocal(rec[:], otr[:, H : H + 1])
                            nc.vector.tensor_scalar_mul(
                                ost[:, t, :], otr[:, 0:H], rec[:]
                            )
                            nc.gpsimd.dma_start(
                                out.ap()[
                                    JBLK * jl + 128 * t : JBLK * jl + 128 * (t + 1),
                                    :,
                                ],
                                ost[:, t, :],
                            )

                    st8 = {}
                    for jl in range(NJ):
                        jg = jglobals[jl]
                        st8[jl] = {"jl": jl, "jg": jg, "nk": 4 * (jg + 1)}

                    def st_step(pair, pbuf, r):
                        rounds = max(st8[x]["nk"] for x in pair)
                        if r < rounds:
                            pbuf[r] = emit_st_pair(st8, pair, r)

                    def pv_step(pair, pbuf, r):
                        if r not in pbuf:
                            return
                        for x in pair:
                            if r < st8[x]["nk"]:
                                emit_pv(st8, pair, x, r, pbuf[r])
                        del pbuf[r]
                        for x in pair:
                            if r == st8[x]["nk"] - 1:
                                epilogue(st8[x]["ot"], x)

                    def alloc_ot(pair):
                        for x in pair:
                            st8[x]["ot"] = bps.tile(
                                [128, JBLK], f32, name=f"ot{vtag}_{x}",
                                tag="ot", bufs=2,
                            )

                    nblk = KVEXT[vtag]  # K/V extent in 512-col blocks
                    small, big = (0, 1), (2, 3)
                    small_rounds = max(st8[x]["nk"] for x in small)
                    big_rounds = max(st8[x]["nk"] for x in big)
                    n_a = small_rounds // 4  # K/V blocks used in phase A

                    # phase A streams read only Qt blocks 0,1 (first qT half);
                    # blocks 2,3 are projected once the first rounds are going
                    proj_block(Qt, qT_sb, 0, 0)
                    proj_block(Qt, qT_sb, 0, 1)
                    alloc_ot(small)
                    pa = {}
                    for b in range(n_a):
                        proj_block(Kt, kT_sb, 1, b)
                        for r in range(4 * b, 4 * b + 4):
                            st_step(small, pa, r)
                        vproj_block(b)
                        if b == 0:
                            proj_block(Qt, qT_sb, 0, 2)
                            proj_block(Qt, qT_sb, 0, 3)
                        for r in range(4 * (b - 1), 4 * b):
                            pv_step(small, pa, r)
                    # seam: start the big pair before draining the small one
                    # so ACT never goes idle across the phase switch
                    alloc_ot(big)
                    pb = {}
                    proj_block(Kt, kT_sb, 1, n_a)
                    for r in range(0, 4):
                        st_step(big, pb, r)
                    for r in range(4 * (n_a - 1), small_rounds):
                        pv_step(small, pa, r)
                    vproj_block(n_a)
                    for r in range(4, 4 * (n_a + 1)):
                        st_step(big, pb, r)
                    cst = 4 * (n_a + 1)
                    cpv = max(0, cst - 4)
                    for r in range(0, cpv):
                        pv_step(big, pb, r)
                    # phase B: big pair chases the remaining K/V stream
                    for b in range(n_a + 1, nblk):
                        proj_block(Kt, kT_sb, 1, b)
                        hi = min(4 * (b + 1), big_rounds)
                        for r in range(cst, hi):
                            st_step(big, pb, r)
                        cst = hi
                        vproj_block(b)
                        pv_hi = max(0, cst - 4)
                        for r in range(cpv, pv_hi):
                            pv_step(big, pb, r)
                        cpv = pv_hi
                    for r in range(cpv, big_rounds):
                        pv_step(big, pb, r)

            with tc.If(pid <= 3) as cmp:
                body(JGLOBALS[0], 0)
            with cmp.Else():
                body(JGLOBALS[1], 1)

    nc.compile()
    return nc


def _get_nc():
    if "nc" not in _CACHE:
        _CACHE["nc"] = _build()
    return _CACHE["nc"]


def _numpy_fallback(query, key, value, Wq, bq, Wk, bk, Wv, bv, mask):
    """Exact reference math in numpy; only used if the mask is not causal."""
    q = np.einsum("bse,he->bsh", query, Wq) + bq
    k = np.einsum("bse,he->bsh", key, Wk) + bk
    v = np.einsum("bse,he->bsh", value, Wv) + bv
    scores = np.einsum("bqh,bkh->bqk", q, k) / np.sqrt(np.float32(query.shape[-1]))
    scores = np.where(np.asarray(mask), scores, -np.inf)
    scores -= scores.max(axis=-1, keepdims=True)
    w = np.exp(scores)
    w /= w.sum(axis=-1, keepdims=True)
    return np.einsum("bqk,bkh->bqh", w, v).astype(np.float32)


def _half_rows(arr_s_first, half):
    """Select this half's query rows (its JGLOBALS blocks) from [S, ...]."""
    return np.concatenate(
        [arr_s_first[JBLK * jg : JBLK * (jg + 1)] for jg in JGLOBALS[half]]
    )


def _prepare_in_maps(query, key, value, Wq, bq, Wk, bk, Wv, bv):
    # Weight columns (and biases) are duplicated into partitions 64..127 so
    # the score matmuls contract over the full 128 partitions (K=64 matmuls
    # never un-throttle the PE clock); scores double, the exp scale halves.
    wT1 = np.stack([Wq.T, Wk.T, Wv.T])
    wT = np.concatenate([wT1, wT1], axis=-1)  # [3, E, 2H]
    # device SBUF layout [p, m, c, h]: partition p = e % 128, chunk c = e//128
    wTp = np.ascontiguousarray(
        wT.reshape(3, 4, 128, 2 * H).transpose(2, 0, 1, 3)
    ).astype(BF16)
    b1 = np.stack([bq, bk, bv]).reshape(3, H)
    bql = np.ascontiguousarray(
        np.concatenate([b1, b1], axis=-1).T
    ).astype(np.float32)  # [2H, 3]
    bvrep = np.tile(
        bv.reshape(1, 1, H).astype(np.float32), (128, 4, 1)
    ).astype(np.float32)
    kT_b = [np.ascontiguousarray(key[b].T).astype(BF16) for b in range(B)]
    vT_b = [np.ascontiguousarray(value[b].T).astype(BF16) for b in range(B)]
    in_maps = []
    for j in range(N_CORES):
        b, half = j % B, j // B
        qslab = _half_rows(query[b], half)
        in_maps.append(
            {
                "qT": np.ascontiguousarray(qslab.T).astype(BF16),
                "kT": kT_b[b],
                "vT": vT_b[b],
                "wTp": wTp,
                "bql": bql,
                "bvrep": bvrep,
            }
        )
    return in_maps


def _assemble(results):
    out = np.empty((B, S, H), np.float32)
    for j in range(N_CORES):
        b, half = j % B, j // B
        r = results[j]["out"]
        for jl, jg in enumerate(JGLOBALS[half]):
            out[b, JBLK * jg : JBLK * (jg + 1)] = r[JBLK * jl : JBLK * (jl + 1)]
    return out


def run(query, key, value, Wq, bq, Wk, bk, Wv, bv, mask, trace=False, **trace_kwargs):
    from concourse.bass_utils import run_bass_kernel_spmd

    mask = np.asarray(mask)
    causal = mask.shape == (1, S, S) and bool(
        np.array_equal(mask[0], np.tril(np.ones((S, S), dtype=bool)))
    )
    if not causal:
        return _numpy_fallback(
            query, key, value, Wq, bq, Wk, bk, Wv, bv, mask
        ), None

    args = [np.asarray(a, np.float32) for a in (query, key, value, Wq, bq, Wk, bk, Wv, bv)]
    nc = _get_nc()
    in_maps = _prepare_in_maps(*args)
    res = run_bass_kernel_spmd(
        nc, in_maps, core_ids=list(range(N_CORES)), trace=trace, **trace_kwargs
    )
    return _assemble(res.results), res


def kernel(query, key, value, Wq, bq, Wk, bk, Wv, bv, mask):
    out, _ = run(query, key, value, Wq, bq, Wk, bk, Wv, bv, mask)
    return out


if __name__ == "__main__":
    rng = np.random.default_rng(0)
    query = rng.standard_normal((B, S, E)).astype(np.float32)
    key = rng.standard_normal((B, S, E)).astype(np.float32)
    value = rng.standard_normal((B, S, E)).astype(np.float32)
    Wq = (rng.standard_normal((H, E)) * 0.02).astype(np.float32)
    Wk = (rng.standard_normal((H, E)) * 0.02).astype(np.float32)
    Wv = (rng.standard_normal((H, E)) * 0.02).astype(np.float32)
    bq = np.zeros(H, np.float32)
    bk = np.zeros(H, np.float32)
    bv = np.zeros(H, np.float32)
    mask = np.tril(np.ones((1, S, S), dtype=bool))
    out = kernel(query, key, value, Wq, bq, Wk, bk, Wv, bv, mask)
    exp = _numpy_fallback(query, key, value, Wq, bq, Wk, bk, Wv, bv, mask)
    err = np.linalg.norm(out - exp) / np.linalg.norm(exp)
    print("self-check rel err:", err)
